# revision 29
# baseline (speedup 1.0000x reference)
"""GRU cell kernel for Trainium2, data-parallel over batch across 8 NeuronCores.

Reference computation (B=8192, D=H=1024), per batch row:
    z = sigmoid(inp@wz + state@uz + bz)
    r = sigmoid(inp@wr + state@ur + br)
    h_ = tanh(inp@wx + bx + (state@wh) * r)
    hid = (1-z)*h_ + state*z

Strategy: each core takes a 1024-row batch shard. HW exec ~120us vs
the 206us fp32r baseline; rel_err 1.67e-2 vs the 2e-2 gate (verified
against a bit-accurate numpy emulation of the quantization scheme).

Phase 1 — the z/r projections fuse into one [1024,2048]@[2048,2048] GEMM
run in fp8(e4m3) with perf_mode=DoubleRow: both operands carry two K-rows
per partition ([K=128, 2, free] APs), so each matmul contracts K=256 in
the same ~229ns a bf16 matmul needs for K=128. Weights are pre-scaled by
32 on the host to clear e4m3's subnormal range; the sigmoid descales for
free via the ACT engine's scale operand, and emits zneg=1-z directly by
negating that scale. fp8 z/r error is damped by sigmoid' <= 1/4. The
phase-1 DVE is otherwise idle, so it precomputes v = z*state; the first
two m-tiles run k-interleaved (2x PE work per arriving k-tile) because
the kernel start is DMA-arrival-limited.

Phase 2 — xh runs in bf16 (its error feeds tanh undamped; fp8 would
break the 2e-2 gate at 2.3e-2) and hh in fp8-DoubleRow (damped by r<1).
Both weight sets are pre-scaled by 32 (exact exponent shift in bf16) so
tanh's scale operand descales the summed PSUM values. The epilogue is 4
ops: t=phh*r+pxh (DVE, PSUM), h=tanh(t/32) (ACT, bf16), u=h*zneg (DVE
2x_1P all-bf16 fast path), out=u+v (GPSIMD, except the last tile where
its latency would stretch the kernel tail). Column-block-split
accumulation makes each block's epilogue overlap the next block's
matmuls, so the post-matmul tail is a single short chain.

Both phases keep the activations stationary and stream weights as the
moving operand, with >=2 matmuls per LDWEIGHTS so weight loads hide
behind compute. All inputs are packed on the host so every DMA moves
2KB-contiguous per-partition lines (phase-2 tensors as one consolidated
DMA each), and everything stays resident in SBUF (~20 MB).
"""

import os
import sys
import types

sys.path.insert(0, "/opt/trn_rl_repo")

import numpy as np
import ml_dtypes

# trace=True under axon needs antenv.axon_hooks, absent from this image.
# Register the same ctypes-backed NTFF hook trn_boot would have installed.
if "antenv.axon_hooks" not in sys.modules:
    _m = types.ModuleType("antenv.axon_hooks")
    _m._hook = None

    def _set_hook(h):
        _m._hook = h

    def _get_hook():
        return _m._hook

    _m.set_axon_ntff_profile_hook = _set_hook
    _m.get_axon_ntff_profile_hook = _get_hook
    sys.modules["antenv.axon_hooks"] = _m
    try:
        from trn_agent_boot.trn_boot import _ntff_profile_via_ctypes

        _m.set_axon_ntff_profile_hook(
            _ntff_profile_via_ctypes("/opt/axon/libaxon_pjrt.so")
        )
    except Exception:
        pass

import concourse.bacc as bacc
import concourse.tile as tile
from concourse import mybir
from concourse.bass_utils import run_bass_kernel_spmd

N_CORES = 8
B, D, H = 8192, 1024, 1024
BL = B // N_CORES  # batch rows per core
P = 128  # partitions
NF = 512  # matmul free dim (one PSUM bank of fp32)
KT = 8  # k-tiles: 8x256 for the fp8 zr GEMM, 8x128 for the bf16 GEMMs
MT = BL // P  # batch m-tiles per core
F32 = mybir.dt.float32
F32R = mybir.dt.float32r
BF16 = mybir.dt.bfloat16
E4 = mybir.dt.float8e4
NP_E4 = ml_dtypes.float8_e4m3
NP_BF = ml_dtypes.bfloat16
WSCALE = 32.0
DR = mybir.MatmulPerfMode.DoubleRow

_CACHE = {}


def _build_program(with_bias):
    nc = bacc.Bacc("TRN2", target_bir_lowering=False, debug=False)

    # [k*128+p, i*2048+c]: acts/weights packed so partition p of k-tile k
    # holds the two 128-row K-subblocks (i=0,1) back to back.
    x8 = nc.declare_dram_parameter("x8", [(D + H) // 2, 2 * BL], E4, isOutput=False)
    wzr = nc.declare_dram_parameter("wzr", [(D + H) // 2, 2 * 2 * H], E4, isOutput=False)
    xbf = nc.declare_dram_parameter("xbf", [D, BL], BF16, isOutput=False)
    wxb = nc.declare_dram_parameter("wxb", [D, H], BF16, isOutput=False)
    wh8 = nc.declare_dram_parameter("wh8", [H // 2, 2 * H], E4, isOutput=False)
    stb = nc.declare_dram_parameter("stb", [BL, H], BF16, isOutput=False)
    if with_bias:
        bzr = nc.declare_dram_parameter("bzr", [1, 2 * H], F32R, isOutput=False)
        bx = nc.declare_dram_parameter("bx", [1, H], F32R, isOutput=False)
    out = nc.declare_dram_parameter("out", [BL, H], F32, isOutput=True)

    with tile.TileContext(nc) as tc:
        with (
            tc.tile_pool(name="acts", bufs=1) as acts,
            tc.tile_pool(name="wres", bufs=1) as wres,
            tc.tile_pool(name="stash", bufs=1) as stash,
            tc.tile_pool(name="tmp", bufs=4) as tmp,
            tc.tile_pool(name="small", bufs=1) as small,
            tc.tile_pool(name="ps", bufs=8, space="PSUM") as ps,
        ):
            # Junk matmuls keep the PE busy while the first input DMAs land:
            # HAM sees sustained activity and un-throttles to 2.4 GHz before
            # the first real matmul issues.
            warm_sb = small.tile([P, 4 * P], F32, tag="warm_sb")
            nc.vector.memset(warm_sb, 0.0)
            warm_ps = ps.tile([P, 4 * P], F32, tag="ps", name="warm_ps")
            for i in range(3):
                nc.tensor.matmul(
                    warm_ps[:, : 2 * P],
                    warm_sb[:, :P],
                    warm_sb[:, : 2 * P],
                    start=True,
                    stop=True,
                )

            if with_bias:
                ones = small.tile([1, P], F32R, tag="ones")
                nc.vector.memset(ones, 1.0)
                bzr_sb = small.tile([1, 2 * H], F32R, tag="bzr")
                nc.sync.dma_start(out=bzr_sb, in_=bzr.ap())
                bx_sb = small.tile([1, H], F32R, tag="bx")
                nc.sync.dma_start(out=bx_sb, in_=bx.ap())

            # ---- resident inputs ----
            # Phase-1 tensors keep per-k-tile DMAs (2KB lines) so the
            # first matmuls start as soon as their k-pair lands; phase-2
            # tensors arrive long before phase 2 starts, so each is one
            # consolidated DMA (fewer completions to drain at kernel end).
            x8_t = [acts.tile([P, 2 * BL], E4, tag=f"x8_{k}", name=f"x8_{k}") for k in range(KT)]
            wzr_t = [wres.tile([P, 4 * H], E4, tag=f"wzr_{k}", name=f"wzr_{k}") for k in range(KT)]
            xbf_a = acts.tile([P, KT * BL], BF16, tag="xbf", name="xbf_a")
            wxb_a = wres.tile([P, KT * H], BF16, tag="wxb", name="wxb_a")
            wh8_a = wres.tile([P, KT * H], E4, tag="wh8", name="wh8_a")
            stb_a = acts.tile([P, MT * H], BF16, tag="stb", name="stb_a")

            # Two hardware DGE queues (SP + Activation) split the input
            # stream: one queue alone sustains ~350 GB/s while the HBM
            # port does ~700, and phase 1 is arrival-limited early on.
            for k in range(KT):
                nc.sync.dma_start(out=x8_t[k], in_=x8.ap()[k * P : (k + 1) * P, :])
                nc.sync.dma_start(out=wzr_t[k], in_=wzr.ap()[k * P : (k + 1) * P, :])
            nc.sync.dma_start(
                out=xbf_a[:, :].rearrange("p (k c) -> p k c", k=KT),
                in_=xbf.ap().rearrange("(k p) c -> p k c", k=KT),
            )
            nc.sync.dma_start(
                out=wxb_a[:, :].rearrange("p (k c) -> p k c", k=KT),
                in_=wxb.ap().rearrange("(k p) c -> p k c", k=KT),
            )
            nc.sync.dma_start(
                out=wh8_a[:, :].rearrange("p (k c) -> p k c", k=KT // 2),
                in_=wh8.ap().rearrange("(k p) c -> p k c", k=KT // 2),
            )
            nc.sync.dma_start(
                out=stb_a[:, :].rearrange("p (m c) -> p m c", m=MT),
                in_=stb.ap().rearrange("(m p) c -> p m c", m=MT),
            )
            xbf_t = [xbf_a[:, k * BL : (k + 1) * BL] for k in range(KT)]
            wxb_t = [wxb_a[:, k * H : (k + 1) * H] for k in range(KT)]
            stb_t = [stb_a[:, m * H : (m + 1) * H] for m in range(MT)]

            # DoubleRow 3D views: [partition, pair, free]
            x8_ap = [t[:, :].rearrange("p (i b) -> p i b", i=2) for t in x8_t]
            wzr_ap = [t[:, :].rearrange("p (i c) -> p i c", i=2) for t in wzr_t]
            wh8_ap = [
                wh8_a[:, 2 * k * H : 2 * (k + 1) * H].rearrange("p (i c) -> p i c", i=2)
                for k in range(KT // 2)
            ]

            # bf16 stashes for all 8 m-tiles: zneg = 1-z, r, and the
            # precomputed v = z*state (so the phase-2 epilogue is just
            # h*zneg + v).
            zn_st = [stash.tile([P, H], BF16, tag=f"zn{m}", name=f"zn{m}") for m in range(MT)]
            r_st = [stash.tile([P, H], BF16, tag=f"r{m}", name=f"r{m}") for m in range(MT)]
            v_st = [stash.tile([P, H], BF16, tag=f"v{m}", name=f"v{m}") for m in range(MT)]

            # ---- Phase 1: fused z/r GEMM in fp8 DoubleRow ----
            # 4 PSUM banks per m-tile (4 moving matmuls per stationary
            # load); two m-tiles pipeline in the 8 banks so the next
            # k-sweep runs while the previous tile's sigmoids drain.
            # The first two m-tiles run k-interleaved as one group: early
            # on the kernel is gated by x8/wzr DMA arrival, and pairing
            # doubles the PE work available per arriving k-tile.
            accs2 = {
                m: [ps.tile([P, NF], F32, tag="ps", name="acc") for _ in range(4)]
                for m in range(2)
            }
            if with_bias:
                for m in range(2):
                    for cb in range(4):
                        nc.tensor.matmul(
                            accs2[m][cb],
                            ones,
                            bzr_sb[:, cb * NF : (cb + 1) * NF],
                            start=True,
                            stop=False,
                        )
            for k in range(KT):
                for m in range(2):
                    lhs = x8_ap[k][:, :, m * P : (m + 1) * P]
                    for cb in range(4):
                        nc.tensor.matmul(
                            accs2[m][cb],
                            lhs,
                            wzr_ap[k][:, :, cb * NF : (cb + 1) * NF],
                            start=(k == 0 and not with_bias),
                            stop=(k == KT - 1),
                            perf_mode=DR,
                        )

            for m in range(MT):
                if m >= 2:
                    accs = [ps.tile([P, NF], F32, tag="ps", name="acc") for _ in range(4)]
                    if with_bias:
                        for cb in range(4):
                            nc.tensor.matmul(
                                accs[cb],
                                ones,
                                bzr_sb[:, cb * NF : (cb + 1) * NF],
                                start=True,
                                stop=False,
                            )
                    for k in range(KT):
                        lhs = x8_ap[k][:, :, m * P : (m + 1) * P]
                        for cb in range(4):
                            nc.tensor.matmul(
                                accs[cb],
                                lhs,
                                wzr_ap[k][:, :, cb * NF : (cb + 1) * NF],
                                start=(k == 0 and not with_bias),
                                stop=(k == KT - 1),
                                perf_mode=DR,
                            )
                else:
                    accs = accs2[m]
                for cb in range(4):
                    neg = cb < 2  # z accumulators produce zneg = 1-z
                    dst = zn_st[m] if neg else r_st[m]
                    csl = slice((cb % 2) * NF, (cb % 2 + 1) * NF)
                    nc.scalar.activation(
                        dst[:, csl],
                        accs[cb],
                        mybir.ActivationFunctionType.Sigmoid,
                        scale=(-1.0 if neg else 1.0) / WSCALE,
                    )
                # v = z*state = state - state*zneg, on the otherwise-idle
                # DVE (all-bf16 ops ride the 2x_1P fast path).
                for cb in range(2):
                    csl = slice(cb * NF, (cb + 1) * NF)
                    qv = tmp.tile([P, NF], BF16, tag="qv", name="qv")
                    nc.vector.tensor_mul(qv, stb_t[m][:, csl], zn_st[m][:, csl])
                    nc.vector.tensor_sub(v_st[m][:, csl], stb_t[m][:, csl], qv)

            # ---- Phase 2: xh (bf16, x32) & hh (fp8 DoubleRow, x32) + fused
            # gate epilogue. Column-block-split: cb0's accumulation
            # finishes before cb1's matmuls start, so cb0's epilogue
            # overlaps cb1's matmuls and the kernel tail is a single
            # epilogue chain.
            for m in range(MT):
                msl = slice(m * P, (m + 1) * P)
                stage = tmp.tile([P, H], F32, tag="stage", name="stage")
                for cb in range(2):
                    csl = slice(cb * NF, (cb + 1) * NF)
                    pxh = ps.tile([P, NF], F32, tag="ps", name="pxh")
                    phh = ps.tile([P, NF], F32, tag="ps", name="phh")
                    if with_bias:
                        nc.tensor.matmul(
                            pxh, ones, bx_sb[:, csl], start=True, stop=False
                        )
                    for k in range(KT):
                        nc.tensor.matmul(
                            pxh,
                            xbf_t[k][:, msl],
                            wxb_t[k][:, csl],
                            start=(k == 0 and not with_bias),
                            stop=(k == KT - 1),
                        )
                    for k in range(KT // 2):
                        nc.tensor.matmul(
                            phh,
                            x8_ap[KT // 2 + k][:, :, msl],
                            wh8_ap[k][:, :, csl],
                            start=(k == 0),
                            stop=(k == KT // 2 - 1),
                            perf_mode=DR,
                        )

                    # PSUM holds 32*xh and 32*hh (weights pre-scaled);
                    # tanh's scale operand descales for free.
                    # h_ = tanh(xh + hh*r); hid = h_*zneg + v
                    # (v = z*state was precomputed in phase 1). The final
                    # add runs on GPSIMD to keep DVE under the matmul
                    # span, except on the last tile where GPSIMD's higher
                    # per-op latency would lengthen the kernel tail. The
                    # last column block runs in 256-col chunks so tanh/DMA
                    # overlap the DVE chain.
                    last = m == MT - 1
                    nq = 2 if (last and cb == 1) else 1
                    cw = NF // nq
                    for qi in range(nq):
                        qs = slice(cb * NF + qi * cw, cb * NF + (qi + 1) * cw)
                        pq = slice(qi * cw, (qi + 1) * cw)
                        t = tmp.tile([P, NF], F32, tag="t", name="t")[:, :cw]
                        h = tmp.tile([P, NF], BF16, tag="h", name="h")[:, :cw]
                        u = tmp.tile([P, NF], BF16, tag="u", name="u")[:, :cw]
                        nc.vector.tensor_mul(t, phh[:, pq], r_st[m][:, qs])
                        nc.vector.tensor_add(t, t, pxh[:, pq])
                        nc.scalar.activation(
                            h, t, mybir.ActivationFunctionType.Tanh,
                            scale=1.0 / WSCALE,
                        )
                        nc.vector.tensor_mul(u, h, zn_st[m][:, qs])
                        eng = nc.vector if (last and cb == 1) else nc.gpsimd
                        eng.tensor_add(stage[:, qs], u, v_st[m][:, qs])
                    if last:
                        nc.sync.dma_start(out=out.ap()[msl, csl], in_=stage[:, csl])
                if m < MT - 1:
                    nc.sync.dma_start(out=out.ap()[msl, :], in_=stage)

    nc.compile()
    return nc


def _get_program(with_bias):
    key = ("nc", with_bias)
    if key not in _CACHE:
        _CACHE[key] = _build_program(with_bias)
    return _CACHE[key]


def kernel(inp, state, wx, bx, wh, wr, ur, uz, wz, br, bz):
    inp = np.asarray(inp, dtype=np.float32)
    state = np.asarray(state, dtype=np.float32)
    wx = np.asarray(wx, np.float32)
    wh = np.asarray(wh, np.float32)

    # fp8 z/r weights: [[wz, wr], [uz, ur]] scaled by 32, packed to
    # [k*128+p, i*2048+c] DoubleRow pair layout.
    w_zr = np.block(
        [
            [np.asarray(wz, np.float32), np.asarray(wr, np.float32)],
            [np.asarray(uz, np.float32), np.asarray(ur, np.float32)],
        ]
    )
    q = np.clip(w_zr * WSCALE, -240.0, 240.0).astype(NP_E4)
    wzr8 = np.ascontiguousarray(
        q.reshape(KT, 2, P, 2 * H).transpose(0, 2, 1, 3).reshape((D + H) // 2, 4 * H)
    )
    # xh weights in bf16 scaled by 32 (exact, exponent shift) and hh
    # weights in fp8 scaled by 32, so both phase-2 PSUM accumulators
    # carry 32x values that tanh descales via its scale operand.
    wxb = np.ascontiguousarray((wx * WSCALE).astype(NP_BF))
    qh = np.clip(wh * WSCALE, -240.0, 240.0).astype(NP_E4)
    wh8 = np.ascontiguousarray(
        qh.reshape(KT // 2, 2, P, H).transpose(0, 2, 1, 3).reshape(H // 2, 2 * H)
    )

    # fp8 activations [inp; state] transposed, DoubleRow pair layout.
    xc = np.concatenate([inp.T, state.T], axis=0)  # [2048, B]
    x8f = (
        np.clip(xc, -240.0, 240.0)
        .astype(NP_E4)
        .reshape(KT, 2, P, B)
        .transpose(0, 2, 1, 3)  # [k, p, i, B]
    )
    xbfT = inp.T.astype(NP_BF)  # [D, B]

    # bzr lands in the PSUM accumulator that the sigmoid descales by
    # 1/WSCALE, so pre-scale it to compensate.
    b_zr = WSCALE * np.concatenate(
        [np.asarray(bz, np.float32), np.asarray(br, np.float32)]
    )[None, :]
    # bx also lands in a 32x-scaled PSUM accumulator.
    b_x = WSCALE * np.ascontiguousarray(np.asarray(bx, np.float32))[None, :]
    with_bias = bool(np.any(b_zr) or np.any(b_x))

    in_maps = []
    for c in range(N_CORES):
        sl = slice(c * BL, (c + 1) * BL)
        im = {
            "x8": np.ascontiguousarray(x8f[:, :, :, sl]).reshape((D + H) // 2, 2 * BL),
            "wzr": wzr8,
            "xbf": np.ascontiguousarray(xbfT[:, sl]),
            "wxb": wxb,
            "wh8": wh8,
            "stb": np.ascontiguousarray(state[sl].astype(NP_BF)),
        }
        if with_bias:
            im["bzr"] = b_zr
            im["bx"] = b_x
        in_maps.append(im)

    nc = _get_program(with_bias)
    trace = bool(int(os.environ.get("GRU_TRACE", "0")))
    res = run_bass_kernel_spmd(nc, in_maps, list(range(N_CORES)), trace=trace)
    if trace:
        _CACHE["last_exec_time_ns"] = res.exec_time_ns
        _CACHE["last_results"] = res
    return np.concatenate([res.results[c]["out"] for c in range(N_CORES)], axis=0)
